# Initial kernel scaffold
#
"""NNUE HalfKP EmbeddingBag + MLP kernel for 8 Trainium2 NeuronCores.

Strategy (data-parallel over the batch):
  - 16384 bags are split into 8 shards of 2048 consecutive bags (one per core).
  - The embedding table (+ a virtual bias1 row) is replicated per core in DRAM.
  - Each core gathers its ~61k rows with gpsimd.dma_gather (1KB rows, 4 SWDGE
    queues), segment-sums them into per-128-bag PSUM blocks with TensorE
    matmuls against on-device-built 0/1 selection matrices, applies
    relu -> fc2 -> relu -> out_w on chip, and writes 2048 floats.
  - int16 gather indices can't address all 41025 rows, so each block's row
    list is split into L (idx < 32768) and H (idx >= 32768) sub-lists with
    different DRAM base pointers.

The SPMD program is shared by all 8 cores, so per-(core,block) row counts are
padded up to the max over cores; pad entries use index -1 (not transferred)
with a leading row-0 pad to keep the valid count >= 1. Per-gather valid
counts are data (loaded into a gpsimd register from SBUF).
"""

import numpy as np

import concourse.bacc as bacc
import concourse.mybir as mybir
from concourse.tile import TileContext
from concourse.masks import make_identity

# ---------------- problem constants (hardcoded per spec) ----------------
NUM_FEATURES = 41024
HIDDEN = 256
FC2 = 32
BATCH = 16384
N_IDX = 491520
N_CORES = 8

BAGS_PER_CORE = BATCH // N_CORES       # 2048
BLOCK_BAGS = 128                       # bags per PSUM block
NBLK = BAGS_PER_CORE // BLOCK_BAGS     # 16
SPLIT = 32768                          # int16 index limit
BIAS_IDX = NUM_FEATURES                # virtual row holding bias1
ROWS_PER_GATHER = 1024                 # HW cap for dma_gather num_idxs
TILE = 128                             # rows per matmul tile
N_QUEUES = 4

TABLE_BF16 = True                     # gather the table in bf16 (halves traffic)


def _ceil_div(a, b):
    return -(-a // b)


def _host_prep(indices, offsets):
    """Build per-core gather lists, bag ids, valid counts and the shared
    chunk schedule."""
    indices = np.asarray(indices).astype(np.int64)
    offsets = np.asarray(offsets).astype(np.int64)
    n = indices.shape[0]
    seg = np.clip(
        np.searchsorted(offsets, np.arange(n), side="right") - 1, 0, BATCH - 1
    )
    # row ranges for each 128-bag block boundary (129*8 boundaries total)
    blk_bounds = np.searchsorted(seg, np.arange(0, BATCH + 1, BLOCK_BAGS))

    # per (core, block): L/H index+bag arrays
    per_cb = []  # [core][block] -> (L_idx, L_bag, H_idx, H_bag)
    for c in range(N_CORES):
        per_b = []
        for b in range(NBLK):
            g = c * NBLK + b
            lo, hi = blk_bounds[g], blk_bounds[g + 1]
            idx_blk = indices[lo:hi]
            bag_blk = seg[lo:hi] - (c * BAGS_PER_CORE + b * BLOCK_BAGS)
            # bias row for each of the 128 bags
            idx_blk = np.concatenate([idx_blk, np.full(BLOCK_BAGS, BIAS_IDX)])
            bag_blk = np.concatenate([bag_blk, np.arange(BLOCK_BAGS)])
            low = idx_blk < SPLIT
            per_b.append(
                (
                    idx_blk[low],
                    bag_blk[low],
                    idx_blk[~low] - SPLIT,
                    bag_blk[~low],
                )
            )
        per_cb.append(per_b)

    # uniform tile counts over cores, per block position
    T_L = [0] * NBLK
    T_H = [0] * NBLK
    for b in range(NBLK):
        for c in range(N_CORES):
            L_idx, _, H_idx, _ = per_cb[c][b]
            T_L[b] = max(T_L[b], _ceil_div(len(L_idx), TILE))
            T_H[b] = max(T_H[b], _ceil_div(max(len(H_idx), 1), TILE))

    # chunk schedule shared by all cores: list of
    # (block, is_low, tiles_in_chunk, tile_col_base, idx_col_base, meta_slot)
    sched = []
    tile_base = 0
    idx_cols = 0
    meta_slot = 0
    for b in range(NBLK):
        t_off = 0
        for is_low, T in ((True, T_L[b]), (False, T_H[b])):
            t0 = 0
            while t0 < T:
                tc = min(ROWS_PER_GATHER // TILE, T - t0)
                sched.append(
                    (b, is_low, tc, tile_base + t_off + t0, idx_cols, meta_slot)
                )
                idx_cols += tc * TILE // 16
                meta_slot += 1
                t0 += tc
            t_off += T
        tile_base += T_L[b] + T_H[b]
    total_tiles = tile_base
    n_gathers = meta_slot

    # per-core blobs
    idx_blobs, bag_blobs, metas = [], [], []
    for c in range(N_CORES):
        idx_blobs.append(np.zeros((128, idx_cols), dtype=np.int16))
        bag_blobs.append(np.full((128, total_tiles), -1.0, dtype=np.float32))
        metas.append(np.zeros((1, n_gathers), dtype=np.int32))

    for c in range(N_CORES):
        idx_arr = idx_blobs[c]
        bag_arr = bag_blobs[c]
        meta = metas[c]
        list_tile_off = {}
        for (b, is_low, tc, tbase, icol, slot) in sched:
            key = (b, is_low)
            t0 = list_tile_off.get(key, 0)
            list_tile_off[key] = t0 + tc
            L_idx, L_bag, H_idx, H_bag = per_cb[c][b]
            lst_idx, lst_bag = (L_idx, L_bag) if is_low else (H_idx, H_bag)
            r0 = t0 * TILE
            r1 = r0 + tc * TILE
            navail = len(lst_idx)
            nvalid = min(max(navail - r0, 0), tc * TILE)
            chunk_idx = np.full(tc * TILE, -1, dtype=np.int64)
            chunk_bag = np.full(tc * TILE, -1.0, dtype=np.float64)
            if nvalid > 0:
                chunk_idx[:nvalid] = lst_idx[r0 : r0 + nvalid]
                chunk_bag[:nvalid] = lst_bag[r0 : r0 + nvalid]
            if nvalid < tc * TILE:
                # keep >=1 valid entry and make the ucode's last-valid scan
                # well defined: first pad slot gathers row 0 (zero-weighted)
                chunk_idx[nvalid] = 0
                nvalid += 1
            meta[0, slot] = nvalid
            # idx wrap: row i -> [i%16, i//16], replicated to 128 partitions
            w = chunk_idx.reshape(tc * TILE // 16, 16).T.astype(np.int16)
            idx_arr[:, icol : icol + tc * TILE // 16] = np.tile(w, (8, 1))
            # bag ids: row j of tile t -> [j, tbase + t]
            bag_arr[:, tbase : tbase + tc] = (
                chunk_bag.reshape(tc, TILE).T.astype(np.float32)
            )

    # chunks whose valid count is tc*TILE on EVERY core can use an immediate
    # num_idxs_reg instead of a per-core register load
    static_full = [
        all(int(metas[c][0, slot]) == sched[i][2] * TILE for c in range(N_CORES))
        for i, slot in enumerate(e[5] for e in sched)
    ]
    sched = [(*e, static_full[i]) for i, e in enumerate(sched)]

    return sched, T_L, T_H, total_tiles, idx_cols, n_gathers, idx_blobs, bag_blobs, metas


def _build_program(sched, T_L, T_H, total_tiles, idx_cols, n_gathers, reps=1):
    tdt = mybir.dt.bfloat16 if TABLE_BF16 else mybir.dt.float32
    f32 = mybir.dt.float32
    nc = bacc.Bacc(
        "TRN2", dynamic_dma_scratch_size=65536, num_swdge_queues=N_QUEUES
    )
    table = nc.dram_tensor(
        "table", [NUM_FEATURES + 1, HIDDEN], tdt, kind="ExternalInput"
    )
    idx_d = nc.dram_tensor("idxs", [128, idx_cols], mybir.dt.int16, kind="ExternalInput")
    bag_d = nc.dram_tensor("bags", [128, total_tiles], f32, kind="ExternalInput")
    meta_d = nc.dram_tensor("meta", [1, n_gathers], mybir.dt.int32, kind="ExternalInput")
    iota_d = nc.dram_tensor("iota", [128, 128], f32, kind="ExternalInput")
    w2_d = nc.dram_tensor("w2", [HIDDEN, FC2], f32, kind="ExternalInput")   # fc2_w.T
    b2_d = nc.dram_tensor("b2", [FC2, 1], f32, kind="ExternalInput")
    w3_d = nc.dram_tensor("w3", [FC2, 1], f32, kind="ExternalInput")        # out_w.T
    b3_d = nc.dram_tensor("b3", [1, 1], f32, kind="ExternalInput")
    out_d = nc.dram_tensor("out", [1, BAGS_PER_CORE], f32, kind="ExternalOutput")

    CHUNK_TILES = ROWS_PER_GATHER // TILE  # 8

    with TileContext(nc) as tc_:
        with (
            tc_.tile_pool(name="const", bufs=1) as cpool,
            tc_.tile_pool(name="gdst", bufs=4) as gpool,
            tc_.tile_pool(name="seg", bufs=4) as spool,
            tc_.tile_pool(name="hrel", bufs=2) as hpool,
            tc_.tile_pool(name="h2", bufs=2) as h2pool,
            tc_.tile_pool(name="ph", bufs=2, space="PSUM") as phpool,
            tc_.tile_pool(name="pt", bufs=2, space="PSUM") as ptpool,
            tc_.tile_pool(name="pm", bufs=2, space="PSUM") as pmpool,
        ):
            idx_sb = cpool.tile([128, idx_cols], mybir.dt.int16)
            bag_sb = cpool.tile([128, total_tiles], f32)
            meta_sb = cpool.tile([1, n_gathers], mybir.dt.int32)
            iota_sb = cpool.tile([128, 128], f32)
            w2_sb = cpool.tile([128, 2 * FC2], f32)
            b2_sb = cpool.tile([FC2, 1], f32)
            w3_sb = cpool.tile([FC2, 1], f32)
            b3_sb = cpool.tile([1, 1], f32)
            ident = cpool.tile([128, 128], f32)
            hT = cpool.tile([128, 2 * BAGS_PER_CORE], f32)
            out_sb = cpool.tile([1, BAGS_PER_CORE], f32)

            nc.sync.dma_start(idx_sb[:, :], idx_d[:, :])
            nc.sync.dma_start(bag_sb[:, :], bag_d[:, :])
            nc.sync.dma_start(meta_sb[:, :], meta_d[:, :])
            nc.sync.dma_start(iota_sb[:, :], iota_d[:, :])
            nc.sync.dma_start(w2_sb[:, 0:FC2], w2_d[0:128, :])
            nc.sync.dma_start(w2_sb[:, FC2 : 2 * FC2], w2_d[128:256, :])
            nc.sync.dma_start(b2_sb[:, :], b2_d[:, :])
            nc.sync.dma_start(w3_sb[:, :], w3_d[:, :])
            nc.sync.dma_start(b3_sb[:, :], b3_d[:, :])
            make_identity(nc, ident[:, :])

            # explicit gather-dst ring, zero-seeded so -1 (skipped) pad rows
            # never feed NaN garbage into the 0-weighted matmul columns
            NRING = 6
            dst_ring = []
            for i in range(NRING):
                t = gpool.tile([128, CHUNK_TILES * HIDDEN], tdt, name=f"dstr{i}", bufs=1)
                nc.vector.memset(t[:, :], 0.0)
                dst_ring.append(t)

            regs = [nc.gpsimd.alloc_register(f"nv{i}") for i in range(4)]

            # group schedule entries by block
            by_block = [[] for _ in range(NBLK)]
            for ent in sched:
                by_block[ent[0]].append(ent)

            gather_state = {"no": 0}

            def one_pass():
                for b in range(NBLK):
                    n_tiles_blk = T_L[b] + T_H[b]
                    psum = phpool.tile([128, HIDDEN], f32, name="psumh", tag="psumh")
                    mm = 0
                    for (_, is_low, tcn, tbase, icol, slot, full) in by_block[b]:
                        gno = gather_state["no"]
                        dst = dst_ring[gno % NRING]
                        src = table[0:SPLIT, :] if is_low else table[SPLIT:, :]
                        reg = regs[gno % 4]
                        nc.gpsimd.reg_load(reg, meta_sb[0:1, slot : slot + 1])
                        nc.gpsimd.dma_gather(
                            dst[:, 0 : tcn * HIDDEN].rearrange(
                                "p (t e) -> p t e", e=HIDDEN
                            ),
                            src,
                            idx_sb[:, icol : icol + tcn * TILE // 16],
                            tcn * TILE,
                            reg,
                            HIDDEN,
                            queue_num=gno % N_QUEUES,
                        )
                        gather_state["no"] = gno + 1
                        seg = spool.tile(
                            [128, CHUNK_TILES * TILE], tdt, name="segt", tag="seg"
                        )
                        for t in range(tcn):
                            nc.vector.tensor_tensor(
                                out=seg[:, t * TILE : (t + 1) * TILE],
                                in0=iota_sb[:, :],
                                in1=bag_sb[
                                    :, tbase + t : tbase + t + 1
                                ].to_broadcast([128, 128]),
                                op=mybir.AluOpType.is_equal,
                            )
                        for t in range(tcn):
                            nc.tensor.matmul(
                                psum[:, :],
                                lhsT=seg[:, t * TILE : (t + 1) * TILE],
                                rhs=dst[:, t * HIDDEN : (t + 1) * HIDDEN],
                                start=(mm == 0),
                                stop=(mm == n_tiles_blk - 1),
                            )
                            mm += 1
                    hrel = hpool.tile([128, HIDDEN], f32, name="hrel", tag="hrel")
                    nc.scalar.activation(
                        hrel[:, :], psum[:, :], mybir.ActivationFunctionType.Relu
                    )
                    for half in range(2):
                        pt = ptpool.tile([128, 128], f32, name="ptt", tag="ptt")
                        nc.tensor.transpose(
                            pt[:, :],
                            hrel[:, half * 128 : (half + 1) * 128],
                            ident[:, :],
                        )
                        nc.vector.tensor_copy(
                            hT[
                                :,
                                half * BAGS_PER_CORE
                                + b * 128 : half * BAGS_PER_CORE
                                + (b + 1) * 128,
                            ],
                            pt[:, :],
                        )

                # tiny MLP over all 2048 bags, 512 at a time
                for g in range(BAGS_PER_CORE // 512):
                    p2 = pmpool.tile([FC2, 512], f32, name="p2t", tag="p2")
                    for half in range(2):
                        nc.tensor.matmul(
                            p2[:, :],
                            lhsT=w2_sb[:, half * FC2 : (half + 1) * FC2],
                            rhs=hT[
                                :,
                                half * BAGS_PER_CORE
                                + g * 512 : half * BAGS_PER_CORE
                                + (g + 1) * 512,
                            ],
                            start=(half == 0),
                            stop=(half == 1),
                        )
                    h2 = h2pool.tile([FC2, 512], f32, name="h2t", tag="h2")
                    nc.scalar.activation(
                        h2[:, :], p2[:, :], mybir.ActivationFunctionType.Relu,
                        bias=b2_sb[:, :],
                    )
                    p3 = pmpool.tile([1, 512], f32, name="p3t", tag="p3")
                    nc.tensor.matmul(
                        p3[:, :], lhsT=w3_sb[:, :], rhs=h2[:, :], start=True, stop=True
                    )
                    nc.vector.tensor_scalar_add(
                        out_sb[:, g * 512 : (g + 1) * 512], p3[:, :], b3_sb[0:1, 0:1]
                    )

            for _rep in range(reps):
                one_pass()
            nc.sync.dma_start(out_d[:, :], out_sb[:, :])
    nc.compile()
    return nc


def _make_in_maps(inputs, sched_data):
    (sched, T_L, T_H, total_tiles, idx_cols, n_gathers,
     idx_blobs, bag_blobs, metas) = sched_data
    embed_weight = np.asarray(inputs["embed_weight"], dtype=np.float32)
    bias1 = np.asarray(inputs["bias1"], dtype=np.float32)
    table_aug = np.concatenate([embed_weight, bias1[None, :]], axis=0)
    if TABLE_BF16:
        import ml_dtypes
        table_aug = table_aug.astype(ml_dtypes.bfloat16)
    fc2_w = np.asarray(inputs["fc2_w"], dtype=np.float32)
    fc2_b = np.asarray(inputs["fc2_b"], dtype=np.float32)
    out_w = np.asarray(inputs["out_w"], dtype=np.float32)
    out_b = np.asarray(inputs["out_b"], dtype=np.float32)
    iota = np.broadcast_to(np.arange(128, dtype=np.float32)[None, :], (128, 128)).copy()
    common = {
        "table": table_aug,
        "iota": iota,
        "w2": fc2_w.T.copy(),
        "b2": fc2_b.reshape(FC2, 1),
        "w3": out_w.reshape(1, FC2).T.copy(),
        "b3": out_b.reshape(1, 1),
    }
    in_maps = []
    for c in range(N_CORES):
        m = dict(common)
        m["idxs"] = idx_blobs[c]
        m["bags"] = bag_blobs[c]
        m["meta"] = metas[c]
        in_maps.append(m)
    return in_maps


def kernel(**inputs) -> np.ndarray:
    from concourse.bass_utils import run_bass_kernel_spmd

    sched_data = _host_prep(inputs["indices"], inputs["offsets"])
    sched, T_L, T_H, total_tiles, idx_cols, n_gathers = sched_data[:6]
    nc = _build_program(sched, T_L, T_H, total_tiles, idx_cols, n_gathers)
    in_maps = _make_in_maps(inputs, sched_data)
    res = run_bass_kernel_spmd(nc, in_maps, core_ids=list(range(N_CORES)))
    out = np.concatenate(
        [res.results[c]["out"].reshape(BAGS_PER_CORE) for c in range(N_CORES)]
    )
    return out.astype(np.float32)



# revision 1
# speedup vs baseline: 1.2375x; 1.2375x over previous
"""NNUE HalfKP EmbeddingBag + MLP kernel for 8 Trainium2 NeuronCores.

Strategy (data-parallel over the batch):
  - 16384 bags are split into 8 shards of 2048 consecutive bags (one per core).
  - The embedding table (+ a virtual bias1 row) is replicated per core in DRAM.
  - Each core gathers its ~61k rows with gpsimd.dma_gather (1KB rows, 4 SWDGE
    queues), segment-sums them into per-128-bag PSUM blocks with TensorE
    matmuls against on-device-built 0/1 selection matrices, applies
    relu -> fc2 -> relu -> out_w on chip, and writes 2048 floats.
  - int16 gather indices can't address all 41025 rows, so each block's row
    list is split into L (idx < 32768) and H (idx >= 32768) sub-lists with
    different DRAM base pointers.

The SPMD program is shared by all 8 cores, so per-(core,block) row counts are
padded up to the max over cores; pad entries use index -1 (not transferred)
with a leading row-0 pad to keep the valid count >= 1. Per-gather valid
counts are data (loaded into a gpsimd register from SBUF).
"""

import numpy as np

import concourse.bacc as bacc
import concourse.mybir as mybir
from concourse.tile import TileContext
from concourse.masks import make_identity

# ---------------- problem constants (hardcoded per spec) ----------------
NUM_FEATURES = 41024
HIDDEN = 256
FC2 = 32
BATCH = 16384
N_IDX = 491520
N_CORES = 8

BAGS_PER_CORE = BATCH // N_CORES       # 2048
BLOCK_BAGS = 128                       # bags per PSUM block
NBLK = BAGS_PER_CORE // BLOCK_BAGS     # 16
SPLIT = 32768                          # int16 index limit
BIAS_IDX = NUM_FEATURES                # virtual row holding bias1
ROWS_PER_GATHER = 1024                 # HW cap for dma_gather num_idxs
TILE = 128                             # rows per matmul tile
N_QUEUES = 4

TABLE_BF16 = True                     # gather the table in bf16 (halves traffic)


def _ceil_div(a, b):
    return -(-a // b)


def _host_prep(indices, offsets):
    """Build per-core gather lists, bag ids, valid counts and the shared
    chunk schedule."""
    indices = np.asarray(indices).astype(np.int64)
    offsets = np.asarray(offsets).astype(np.int64)
    n = indices.shape[0]
    seg = np.clip(
        np.searchsorted(offsets, np.arange(n), side="right") - 1, 0, BATCH - 1
    )
    # row ranges for each 128-bag block boundary (129*8 boundaries total)
    blk_bounds = np.searchsorted(seg, np.arange(0, BATCH + 1, BLOCK_BAGS))

    # per (core, block): L/H index+bag arrays
    per_cb = []  # [core][block] -> (L_idx, L_bag, H_idx, H_bag)
    for c in range(N_CORES):
        per_b = []
        for b in range(NBLK):
            g = c * NBLK + b
            lo, hi = blk_bounds[g], blk_bounds[g + 1]
            idx_blk = indices[lo:hi]
            bag_blk = seg[lo:hi] - (c * BAGS_PER_CORE + b * BLOCK_BAGS)
            # bias row for each of the 128 bags
            idx_blk = np.concatenate([idx_blk, np.full(BLOCK_BAGS, BIAS_IDX)])
            bag_blk = np.concatenate([bag_blk, np.arange(BLOCK_BAGS)])
            low = idx_blk < SPLIT
            per_b.append(
                (
                    idx_blk[low],
                    bag_blk[low],
                    idx_blk[~low] - SPLIT,
                    bag_blk[~low],
                )
            )
        per_cb.append(per_b)

    # uniform tile counts over cores, per block position
    T_L = [0] * NBLK
    T_H = [0] * NBLK
    for b in range(NBLK):
        for c in range(N_CORES):
            L_idx, _, H_idx, _ = per_cb[c][b]
            T_L[b] = max(T_L[b], _ceil_div(len(L_idx), TILE))
            T_H[b] = max(T_H[b], _ceil_div(max(len(H_idx), 1), TILE))

    # chunk schedule shared by all cores: list of
    # (block, is_low, tiles_in_chunk, tile_col_base, idx_col_base, meta_slot)
    sched = []
    tile_base = 0
    idx_cols = 0
    meta_slot = 0
    for b in range(NBLK):
        t_off = 0
        for is_low, T in ((True, T_L[b]), (False, T_H[b])):
            t0 = 0
            while t0 < T:
                tc = min(ROWS_PER_GATHER // TILE, T - t0)
                sched.append(
                    (b, is_low, tc, tile_base + t_off + t0, idx_cols, meta_slot)
                )
                idx_cols += tc * TILE // 16
                meta_slot += 1
                t0 += tc
            t_off += T
        tile_base += T_L[b] + T_H[b]
    total_tiles = tile_base
    n_gathers = meta_slot

    # per-core blobs
    idx_blobs, bag_blobs, metas = [], [], []
    for c in range(N_CORES):
        idx_blobs.append(np.zeros((128, idx_cols), dtype=np.int16))
        bag_blobs.append(np.full((128, total_tiles), -1.0, dtype=np.float32))
        metas.append(np.zeros((1, n_gathers), dtype=np.int32))

    for c in range(N_CORES):
        idx_arr = idx_blobs[c]
        bag_arr = bag_blobs[c]
        meta = metas[c]
        list_tile_off = {}
        for (b, is_low, tc, tbase, icol, slot) in sched:
            key = (b, is_low)
            t0 = list_tile_off.get(key, 0)
            list_tile_off[key] = t0 + tc
            L_idx, L_bag, H_idx, H_bag = per_cb[c][b]
            lst_idx, lst_bag = (L_idx, L_bag) if is_low else (H_idx, H_bag)
            r0 = t0 * TILE
            r1 = r0 + tc * TILE
            navail = len(lst_idx)
            nvalid = min(max(navail - r0, 0), tc * TILE)
            chunk_idx = np.full(tc * TILE, -1, dtype=np.int64)
            chunk_bag = np.full(tc * TILE, -1.0, dtype=np.float64)
            if nvalid > 0:
                chunk_idx[:nvalid] = lst_idx[r0 : r0 + nvalid]
                chunk_bag[:nvalid] = lst_bag[r0 : r0 + nvalid]
            if nvalid < tc * TILE:
                # keep >=1 valid entry and make the ucode's last-valid scan
                # well defined: first pad slot gathers row 0 (zero-weighted)
                chunk_idx[nvalid] = 0
                nvalid += 1
            meta[0, slot] = nvalid
            # idx wrap: row i -> [i%16, i//16], replicated to 128 partitions
            w = chunk_idx.reshape(tc * TILE // 16, 16).T.astype(np.int16)
            idx_arr[:, icol : icol + tc * TILE // 16] = np.tile(w, (8, 1))
            # bag ids: row j of tile t -> [j, tbase + t]
            bag_arr[:, tbase : tbase + tc] = (
                chunk_bag.reshape(tc, TILE).T.astype(np.float32)
            )

    # chunks whose valid count is tc*TILE on EVERY core can use an immediate
    # num_idxs_reg instead of a per-core register load
    static_full = [
        all(int(metas[c][0, slot]) == sched[i][2] * TILE for c in range(N_CORES))
        for i, slot in enumerate(e[5] for e in sched)
    ]
    sched = [(*e, static_full[i]) for i, e in enumerate(sched)]

    return sched, T_L, T_H, total_tiles, idx_cols, n_gathers, idx_blobs, bag_blobs, metas


def _build_program(sched, T_L, T_H, total_tiles, idx_cols, n_gathers, reps=1):
    tdt = mybir.dt.bfloat16 if TABLE_BF16 else mybir.dt.float32
    f32 = mybir.dt.float32
    nc = bacc.Bacc(
        "TRN2", dynamic_dma_scratch_size=65536, num_swdge_queues=N_QUEUES
    )
    table = nc.dram_tensor(
        "table", [NUM_FEATURES + 1, HIDDEN], tdt, kind="ExternalInput"
    )
    idx_d = nc.dram_tensor("idxs", [128, idx_cols], mybir.dt.int16, kind="ExternalInput")
    bag_d = nc.dram_tensor("bags", [128, total_tiles], f32, kind="ExternalInput")
    meta_d = nc.dram_tensor("meta", [1, n_gathers], mybir.dt.int32, kind="ExternalInput")
    iota_d = nc.dram_tensor("iota", [128, 128], f32, kind="ExternalInput")
    w2_d = nc.dram_tensor("w2", [HIDDEN, FC2], f32, kind="ExternalInput")   # fc2_w.T
    b2_d = nc.dram_tensor("b2", [FC2, 1], f32, kind="ExternalInput")
    w3_d = nc.dram_tensor("w3", [FC2, 1], f32, kind="ExternalInput")        # out_w.T
    b3_d = nc.dram_tensor("b3", [1, 1], f32, kind="ExternalInput")
    out_d = nc.dram_tensor("out", [1, BAGS_PER_CORE], f32, kind="ExternalOutput")

    CHUNK_TILES = ROWS_PER_GATHER // TILE  # 8

    with TileContext(nc) as tc_:
        with (
            tc_.tile_pool(name="const", bufs=1) as cpool,
            tc_.tile_pool(name="gdst", bufs=4) as gpool,
            tc_.tile_pool(name="seg", bufs=4) as spool,
            tc_.tile_pool(name="hrel", bufs=2) as hpool,
            tc_.tile_pool(name="h2", bufs=2) as h2pool,
            tc_.tile_pool(name="ph", bufs=2, space="PSUM") as phpool,
            tc_.tile_pool(name="pt", bufs=2, space="PSUM") as ptpool,
            tc_.tile_pool(name="pm", bufs=2, space="PSUM") as pmpool,
        ):
            idx_sb = cpool.tile([128, idx_cols], mybir.dt.int16)
            bag_sb = cpool.tile([128, total_tiles], f32)
            meta_sb = cpool.tile([1, n_gathers], mybir.dt.int32)
            iota_sb = cpool.tile([128, 128], f32)
            w2_sb = cpool.tile([128, 2 * FC2], f32)
            b2_sb = cpool.tile([FC2, 1], f32)
            w3_sb = cpool.tile([FC2, 1], f32)
            b3_sb = cpool.tile([1, 1], f32)
            ident = cpool.tile([128, 128], f32)
            hT = cpool.tile([128, 2 * BAGS_PER_CORE], f32)
            out_sb = cpool.tile([1, BAGS_PER_CORE], f32)

            nc.sync.dma_start(idx_sb[:, :], idx_d[:, :])
            nc.sync.dma_start(bag_sb[:, :], bag_d[:, :])
            nc.sync.dma_start(meta_sb[:, :], meta_d[:, :])
            nc.sync.dma_start(iota_sb[:, :], iota_d[:, :])
            nc.sync.dma_start(w2_sb[:, 0:FC2], w2_d[0:128, :])
            nc.sync.dma_start(w2_sb[:, FC2 : 2 * FC2], w2_d[128:256, :])
            nc.sync.dma_start(b2_sb[:, :], b2_d[:, :])
            nc.sync.dma_start(w3_sb[:, :], w3_d[:, :])
            nc.sync.dma_start(b3_sb[:, :], b3_d[:, :])
            make_identity(nc, ident[:, :])

            # explicit gather-dst ring, zero-seeded so -1 (skipped) pad rows
            # never feed NaN garbage into the 0-weighted matmul columns
            NRING = 6
            dst_ring = []
            for i in range(NRING):
                t = gpool.tile([128, CHUNK_TILES * HIDDEN], tdt, name=f"dstr{i}", bufs=1)
                nc.vector.memset(t[:, :], 0.0)
                dst_ring.append(t)

            regs = [nc.gpsimd.alloc_register(f"nv{i}") for i in range(4)]

            # group schedule entries by block
            by_block = [[] for _ in range(NBLK)]
            for ent in sched:
                by_block[ent[0]].append(ent)

            gather_state = {"no": 0}

            def one_pass():
                for b in range(NBLK):
                    n_tiles_blk = T_L[b] + T_H[b]
                    psum = phpool.tile([128, HIDDEN], f32, name="psumh", tag="psumh")
                    mm = 0
                    for (_, is_low, tcn, tbase, icol, slot, full) in by_block[b]:
                        gno = gather_state["no"]
                        dst = dst_ring[gno % NRING]
                        src = table[0:SPLIT, :] if is_low else table[SPLIT:, :]
                        reg = regs[gno % 4]
                        nc.gpsimd.reg_load(reg, meta_sb[0:1, slot : slot + 1])
                        nc.gpsimd.dma_gather(
                            dst[:, 0 : tcn * HIDDEN].rearrange(
                                "p (t e) -> p t e", e=HIDDEN
                            ),
                            src,
                            idx_sb[:, icol : icol + tcn * TILE // 16],
                            tcn * TILE,
                            reg,
                            HIDDEN,
                            queue_num=gno % N_QUEUES,
                        )
                        gather_state["no"] = gno + 1
                        seg = spool.tile(
                            [128, CHUNK_TILES * TILE], tdt, name="segt", tag="seg"
                        )
                        for t in range(tcn):
                            nc.vector.tensor_tensor(
                                out=seg[:, t * TILE : (t + 1) * TILE],
                                in0=iota_sb[:, :],
                                in1=bag_sb[
                                    :, tbase + t : tbase + t + 1
                                ].to_broadcast([128, 128]),
                                op=mybir.AluOpType.is_equal,
                            )
                        for t in range(tcn):
                            nc.tensor.matmul(
                                psum[:, :],
                                lhsT=seg[:, t * TILE : (t + 1) * TILE],
                                rhs=dst[:, t * HIDDEN : (t + 1) * HIDDEN],
                                start=(mm == 0),
                                stop=(mm == n_tiles_blk - 1),
                            )
                            mm += 1
                    hrel = hpool.tile([128, HIDDEN], f32, name="hrel", tag="hrel")
                    nc.scalar.activation(
                        hrel[:, :], psum[:, :], mybir.ActivationFunctionType.Relu
                    )
                    for half in range(2):
                        pt = ptpool.tile([128, 128], f32, name="ptt", tag="ptt")
                        nc.tensor.transpose(
                            pt[:, :],
                            hrel[:, half * 128 : (half + 1) * 128],
                            ident[:, :],
                        )
                        nc.vector.tensor_copy(
                            hT[
                                :,
                                half * BAGS_PER_CORE
                                + b * 128 : half * BAGS_PER_CORE
                                + (b + 1) * 128,
                            ],
                            pt[:, :],
                        )

                # tiny MLP over all 2048 bags, 512 at a time
                for g in range(BAGS_PER_CORE // 512):
                    p2 = pmpool.tile([FC2, 512], f32, name="p2t", tag="p2")
                    for half in range(2):
                        nc.tensor.matmul(
                            p2[:, :],
                            lhsT=w2_sb[:, half * FC2 : (half + 1) * FC2],
                            rhs=hT[
                                :,
                                half * BAGS_PER_CORE
                                + g * 512 : half * BAGS_PER_CORE
                                + (g + 1) * 512,
                            ],
                            start=(half == 0),
                            stop=(half == 1),
                        )
                    h2 = h2pool.tile([FC2, 512], f32, name="h2t", tag="h2")
                    nc.scalar.activation(
                        h2[:, :], p2[:, :], mybir.ActivationFunctionType.Relu,
                        bias=b2_sb[:, :],
                    )
                    p3 = pmpool.tile([1, 512], f32, name="p3t", tag="p3")
                    nc.tensor.matmul(
                        p3[:, :], lhsT=w3_sb[:, :], rhs=h2[:, :], start=True, stop=True
                    )
                    nc.vector.tensor_scalar_add(
                        out_sb[:, g * 512 : (g + 1) * 512], p3[:, :], b3_sb[0:1, 0:1]
                    )

            for _rep in range(reps):
                one_pass()
            nc.sync.dma_start(out_d[:, :], out_sb[:, :])
    nc.compile()
    return nc


def _make_in_maps(inputs, sched_data):
    (sched, T_L, T_H, total_tiles, idx_cols, n_gathers,
     idx_blobs, bag_blobs, metas) = sched_data
    embed_weight = np.asarray(inputs["embed_weight"], dtype=np.float32)
    bias1 = np.asarray(inputs["bias1"], dtype=np.float32)
    table_aug = np.concatenate([embed_weight, bias1[None, :]], axis=0)
    if TABLE_BF16:
        import ml_dtypes
        table_aug = table_aug.astype(ml_dtypes.bfloat16)
    fc2_w = np.asarray(inputs["fc2_w"], dtype=np.float32)
    fc2_b = np.asarray(inputs["fc2_b"], dtype=np.float32)
    out_w = np.asarray(inputs["out_w"], dtype=np.float32)
    out_b = np.asarray(inputs["out_b"], dtype=np.float32)
    iota = np.broadcast_to(np.arange(128, dtype=np.float32)[None, :], (128, 128)).copy()
    common = {
        "table": table_aug,
        "iota": iota,
        "w2": fc2_w.T.copy(),
        "b2": fc2_b.reshape(FC2, 1),
        "w3": out_w.reshape(1, FC2).T.copy(),
        "b3": out_b.reshape(1, 1),
    }
    in_maps = []
    for c in range(N_CORES):
        m = dict(common)
        m["idxs"] = idx_blobs[c]
        m["bags"] = bag_blobs[c]
        m["meta"] = metas[c]
        in_maps.append(m)
    return in_maps


def kernel(**inputs) -> np.ndarray:
    from concourse.bass_utils import run_bass_kernel_spmd

    sched_data = _host_prep(inputs["indices"], inputs["offsets"])
    sched, T_L, T_H, total_tiles, idx_cols, n_gathers = sched_data[:6]
    nc = _build_program(sched, T_L, T_H, total_tiles, idx_cols, n_gathers)
    in_maps = _make_in_maps(inputs, sched_data)
    res = run_bass_kernel_spmd(nc, in_maps, core_ids=list(range(N_CORES)))
    out = np.concatenate(
        [res.results[c]["out"].reshape(BAGS_PER_CORE) for c in range(N_CORES)]
    )
    return out.astype(np.float32)

